# revision 50
# baseline (speedup 1.0000x reference)
"""Trainium2 Bass kernel for nn_MHABlock (dense transformer block).

Sharding: data-parallel over batch — 8 cores x 4 batches (2048 tokens/core).
BatchNorm stats are exact via two cross-core AllGathers of per-core
(mean, var) pairs ([128,2] each) + a local reduction.

On-device layout is E-major ("hT" = [E(128 partitions), tokens(free)]).
Attention uses the "scoresT" formulation (scores transposed: [k_tok, q_tok]):
softmax sums arrive free from an all-ones column prepended to V (row 32*hh of
the attnV PSUM = sum_k exp). Scores run in float32r (rounded into qT/kT by
the PSUM-evacuation copies); embedding/q/k/v/attnV/FFN in bf16. Softmax
normalization broadcasts 1/sum across each head's 32-partition group with a
single block-diagonal matmul (E_blk) + one tensor_tensor multiply; V_aug pad
columns are 1.0 so every reciprocal stays finite. Phase A is drizzled into
the attention stages (stage b only needs q/k chunk b + V tiles 4b..4b+3);
attnV lags scores by one chunk so the PE queue never head-of-line blocks on
exp. BN stats use bn_stats/bn_aggr per 512-token chunk overlapped with
attention/FFN; cross-core reduction is an AllGather of (mean, E[x^2]) pairs
+ local reduce (cheaper than AllReduce in both model and practice). Dummy
"warmer" matmuls keep the PE p-state up through the BN1 collective, and the
BN1 scale is folded into fW1 (fW1s) so FFN1 starts straight from raw h1.
"""

import numpy as np

B, N, D_IN, E, H, KD, FF = 32, 512, 2, 128, 8, 16, 512
NCORES = 8
BPC = B // NCORES          # batches per core
T = BPC * N                # 2048 local tokens
NTOK = B * N               # global token count for BN
NORM = 1.0 / np.sqrt(16.0)
EPS = 1e-5

_CACHE = {}
LAST_RESULT = None


def _build_nc():
    import concourse.bass as bass  # noqa: F401
    import concourse.mybir as mybir
    import concourse.tile as tile
    from concourse import bacc

    f32 = mybir.dt.float32
    f32r = mybir.dt.float32r
    bf16 = mybir.dt.bfloat16
    Act = mybir.ActivationFunctionType
    Alu = mybir.AluOpType
    AX = mybir.AxisListType

    nc = bacc.Bacc("TRN2", target_bir_lowering=False, debug=False,
                   enable_asserts=False, num_devices=NCORES)

    # ---- DRAM I/O ----
    d_xA = nc.dram_tensor("xA", [3, T], bf16, kind="ExternalInput").ap()
    d_WeA = nc.dram_tensor("WeA", [3, E], bf16, kind="ExternalInput").ap()
    d_WqQ = nc.dram_tensor("WqQ", [E, 256], bf16, kind="ExternalInput").ap()
    d_WkQ = nc.dram_tensor("WkQ", [E, 256], bf16, kind="ExternalInput").ap()
    d_WvI = nc.dram_tensor("WvI", [E, 128], bf16, kind="ExternalInput").ap()
    d_WoQ = nc.dram_tensor("WoQ", [128, 256], bf16, kind="ExternalInput").ap()
    d_fW1 = nc.dram_tensor("fW1", [E, FF], bf16, kind="ExternalInput").ap()
    d_fW2 = nc.dram_tensor("fW2", [128, 512], bf16, kind="ExternalInput").ap()
    d_Eblk = nc.dram_tensor("Eblk", [128, 128], bf16, kind="ExternalInput").ap()
    d_vecs = nc.dram_tensor("vecs", [128, 12], f32, kind="ExternalInput").ap()
    d_yT = nc.dram_tensor("yT", [E, T], f32, kind="ExternalOutput").ap()

    RG = [list(range(NCORES))]

    with tile.TileContext(nc) as tc:
        with tc.sbuf_pool(name="sb", bufs=1) as sb, \
             tc.psum_pool(name="ps", bufs=1) as ps, \
             tc.tile_pool(name="dr", bufs=1, space="DRAM") as dr:

            def P(shape, dt, name):  # persistent tile
                return sb.tile(shape, dt, name=name, tag=name, bufs=1)

            xA = P([3, T], bf16, "xA_sb")
            WeA_sb = P([3, E], bf16, "WeA_sb")
            WqQ_sb = P([128, 256], bf16, "WqQ_sb")
            WkQ_sb = P([128, 256], bf16, "WkQ_sb")
            WvI_sb = P([128, 128], bf16, "WvI_sb")
            WoQ_sb = P([128, 256], bf16, "WoQ_sb")
            fW1_sb = P([128, FF], bf16, "fW1_sb")
            fW2_sb = P([128, 512], bf16, "fW2_sb")
            Eblk_sb = P([128, 128], bf16, "Eblk_sb")
            Eblkf_sb = P([128, 128], f32, "Eblkf_sb")
            vecs_sb = P([128, 12], f32, "vecs_sb")

            H0f = P([128, T], f32, "H0f")
            H0b = P([128, T], bf16, "H0b")
            qT = [P([128, T], f32r, f"qT{g}") for g in range(2)]
            kT = [P([128, T], f32r, f"kT{g}") for g in range(2)]
            V_aug = P([128, 16 * 256], bf16, "V_aug")
            h1T = P([128, T], f32, "h1T")
            h1b = P([128, T], bf16, "h1b")
            h1nb = P([128, T], bf16, "h1nb")
            fW1s = P([128, FF], bf16, "fW1s")
            fbias = P([128, 4], f32, "fbias")
            h2T = [P([128, T], bf16, f"h2T{qf}") for qf in range(4)]
            yT = P([128, T], f32, "yT_sb")
            recs = P([128, 512], f32, "recs")
            recsb = P([128, 512], bf16, "recsb")
            bns1 = P([128, 24], f32, "bns1")
            bns2 = P([128, 24], f32, "bns2")
            gst1 = P([128, 16], f32, "gst1")
            gst2 = P([128, 16], f32, "gst2")
            st = P([128, 12], f32, "st")    # scratch for BN math

            # ---- load inputs ----
            nc.sync.dma_start(xA[:], d_xA)
            nc.sync.dma_start(WeA_sb[:], d_WeA)
            nc.sync.dma_start(WqQ_sb[:], d_WqQ)
            nc.sync.dma_start(WkQ_sb[:], d_WkQ)
            nc.sync.dma_start(WvI_sb[:], d_WvI)
            nc.sync.dma_start(WoQ_sb[:], d_WoQ)
            nc.sync.dma_start(fW1_sb[:], d_fW1)
            nc.sync.dma_start(fW2_sb[:], d_fW2)
            nc.sync.dma_start(Eblk_sb[:], d_Eblk)
            nc.gpsimd.tensor_copy(Eblkf_sb[:], Eblk_sb[:])
            nc.sync.dma_start(vecs_sb[:], d_vecs)
            nc.gpsimd.memset(V_aug[:], 0.0)
            # ones at w=0 (softmax sums) and w=17..31 (keeps reciprocals finite)
            va = V_aug.rearrange("p (t h w) -> p (t h) w", t=16, h=8)
            nc.gpsimd.memset(va[:, :, 0:1], 1.0)
            nc.gpsimd.memset(va[:, :, 17:32], 1.0)

            # ---- Phase A helpers (emitted piecewise, interleaved with B) ----
            def emit_embed(c):
                pm = ps.tile([128, 512], f32, tag="mm", bufs=2, name=f"pe{c}")
                nc.tensor.matmul(pm[:], lhsT=WeA_sb[:],
                                 rhs=xA[:, 512 * c:512 * (c + 1)],
                                 start=True, stop=True)
                lo, hi = 512 * c, 512 * (c + 1)
                nc.vector.tensor_copy(H0b[:, lo:hi], pm[:])
                nc.scalar.copy(H0f[:, lo:hi], pm[:])

            def emit_v(t):
                pv = ps.tile([128, 128], f32, tag="mm", bufs=2, name=f"pv{t}")
                nc.tensor.matmul(pv[:], lhsT=H0b[:, 128 * t:128 * (t + 1)],
                                 rhs=WvI_sb[:], start=True, stop=True)
                dst = V_aug[:, 256 * t:256 * (t + 1)]
                dst = dst.rearrange("p (h w) -> p h w", h=8)[:, :, 1:17]
                src = pv.rearrange("p (h w) -> p h w", h=8)
                nc.vector.tensor_copy(dst, src)

            def emit_qk_chunk(g, c):
                # q/k chunks go through the "mm" ring to stay out of the
                # attention pipeline's "sc" ring
                for W, dstT in ((WqQ_sb, qT), (WkQ_sb, kT)):
                    pq = ps.tile([128, 512], f32, tag="mm", bufs=2,
                                 name=f"pqk{g}{c}_{0 if W is WqQ_sb else 1}")
                    nc.tensor.matmul(pq[:],
                                     lhsT=W[:, 128 * g:128 * (g + 1)],
                                     rhs=H0b[:, 512 * c:512 * (c + 1)],
                                     start=True, stop=True)
                    nc.vector.tensor_copy(
                        dstT[g][:, 512 * c:512 * (c + 1)], pq[:])

            # minimal prefix needed to start attention on (b0, g0/g1):
            # stage b only touches q/k chunk b and V tiles 4b..4b+3
            emit_embed(0)
            emit_qk_chunk(0, 0)
            emit_qk_chunk(1, 0)
            for t in range(4):
                emit_v(t)
            # deferred phase-A work, drizzled into the attention stages
            deferred_a = []
            for c in range(1, 4):
                deferred_a.append(lambda c=c: emit_embed(c))
                deferred_a.append(lambda c=c: emit_qk_chunk(0, c))
                deferred_a.append(lambda c=c: emit_qk_chunk(1, c))
                deferred_a.extend(lambda t=t: emit_v(t)
                                  for t in range(4 * c, 4 * c + 4))

            # ---- Phase B: attention (software-pipelined stages) ----
            HT = {}

            def emit_norm(b, g, av, last=False):
                nc.vector.reciprocal_approx_fast(recs[:], av[:])
                if not last:
                    nc.gpsimd.tensor_copy(recsb[:], recs[:])
                raw = sb.tile([128, 512], f32, tag="raw", bufs=2,
                              name=f"raw{b}{g}")
                nc.vector.tensor_copy(raw[:], av[:])
                Rp = ps.tile([128, 512], f32, tag="mm", bufs=2,
                             name=f"Rp{b}{g}")
                if last:
                    # fp32 matmul (4 cyc/row) but skips the Pool-hop latency —
                    # this one sits on the BN1 critical tail where PE is idle
                    nc.tensor.matmul(Rp[:], lhsT=Eblkf_sb[:], rhs=recs[:],
                                     start=True, stop=True)
                else:
                    nc.tensor.matmul(Rp[:], lhsT=Eblk_sb[:], rhs=recsb[:],
                                     start=True, stop=True)
                HT[(b, g)] = sb.tile([128, 512], bf16, tag=f"HT{g}", bufs=2,
                                     name=f"HT{g}_{b}")
                nc.vector.tensor_mul(HT[(b, g)][:], raw[:], Rp[:])

            def emit_po(b):
                po = ps.tile([128, 512], f32, tag="mm", bufs=2, name=f"po{b}")
                for g in range(2):
                    nc.tensor.matmul(po[:],
                                     lhsT=WoQ_sb[:, 128 * g:128 * (g + 1)],
                                     rhs=HT[(b, g)][:],
                                     start=(g == 0), stop=(g == 1))
                nc.vector.tensor_add(h1T[:, 512 * b:512 * (b + 1)], po[:],
                                     H0f[:, 512 * b:512 * (b + 1)])
                nc.vector.bn_stats(bns1[:, 6 * b:6 * (b + 1)],
                                   h1T[:, 512 * b:512 * (b + 1)])
                nc.vector.tensor_copy(h1b[:, 512 * b:512 * (b + 1)],
                                      h1T[:, 512 * b:512 * (b + 1)])

            stages = [(b, g) for b in range(4) for g in range(2)]
            pending = []          # [(kind, args), ...] from previous stage
            for k, (b, g) in enumerate(stages):
                av = ps.tile([128, 512], f32, tag="av", bufs=2,
                             name=f"av{b}{g}")
                def emit_av(c, exs):
                    for hp in range(2):
                        for j in range(2):
                            hh = 2 * hp + j
                            h = 4 * g + hh
                            tci = 4 * b + c
                            nc.tensor.matmul(
                                av[32 * hh:32 * (hh + 1), :],
                                lhsT=V_aug[:, 256 * tci + 32 * h:
                                           256 * tci + 32 * h + 32],
                                rhs=exs[hp][:, 512 * j:512 * (j + 1)],
                                start=(c == 0), stop=(c == 3),
                                tile_position=(0, 32 * hh))

                exq = {}
                for c in range(4):
                    for hp in range(2):
                        scp = ps.tile([128, 1024], f32, tag="sc", bufs=2,
                                      name=f"scp{b}{g}{c}{hp}")
                        for j in range(2):
                            hh = 2 * hp + j
                            nc.tensor.matmul(
                                scp[:, 512 * j:512 * (j + 1)],
                                lhsT=kT[g][32 * hh:32 * (hh + 1),
                                           512 * b + 128 * c:
                                           512 * b + 128 * (c + 1)],
                                rhs=qT[g][32 * hh:32 * (hh + 1),
                                          512 * b:512 * (b + 1)],
                                start=True, stop=True,
                                tile_position=(32 * hh, 0))
                        ex = sb.tile([128, 1024], bf16, tag="ex", bufs=6,
                                     name=f"ex{b}{g}{c}{hp}")
                        nc.scalar.activation(ex[:], scp[:], Act.Exp,
                                             scale=float(NORM))
                        exq[(c, hp)] = ex
                    # attnV for the PREVIOUS chunk: keeps the next chunk's
                    # score MMs ahead of the exp dependency in the PE queue
                    if c >= 1:
                        emit_av(c - 1, (exq[(c - 1, 0)], exq[(c - 1, 1)]))
                    if c == 1:
                        for fn in pending:
                            fn()
                        pending = []
                    if c in (1, 2):
                        for _ in range(2):
                            if deferred_a:
                                deferred_a.pop(0)()
                emit_av(3, (exq[(3, 0)], exq[(3, 1)]))
                # queue this stage's normalization (and po once g==1 done)
                pending = [lambda b=b, g=g, av=av, lt=(k == len(stages) - 1):
                           emit_norm(b, g, av, last=lt)]
                if g == 1:
                    pending.append(lambda b=b: emit_po(b))
            for fn in pending:
                fn()
            for fn in deferred_a:
                fn()

            # ---- BN sync helper: AllGather of per-core (mean, E[x^2]) ----
            def bn_sync(bns, gst, agcol, scol, ccname):
                agg = st[:, agcol:agcol + 2]
                nc.vector.bn_aggr(agg, bns[:])
                scr = st[:, scol:scol + 1]
                nc.vector.tensor_mul(scr, agg[:, 0:1], agg[:, 0:1])
                nc.vector.tensor_add(agg[:, 1:2], agg[:, 1:2], scr)
                cc_in = dr.tile([128, 2], f32, name=f"{ccname}_in",
                                tag=f"{ccname}_in")
                cc_out = dr.tile([1024, 2], f32, addr_space="Shared",
                                 name=f"{ccname}_out", tag=f"{ccname}_out")
                nc.sync.dma_start(cc_in[:], agg)
                nc.gpsimd.collective_compute(
                    "AllGather", Alu.bypass, replica_groups=RG,
                    ins=[cc_in[:]], outs=[cc_out[:]])
                src = cc_out[:].rearrange("(s p) j -> p s j", s=8)
                dst = gst[:].rearrange("p (s j) -> p s j", s=8)
                nc.sync.dma_start(dst, src)

            def bn_affine(gst, bnv, wcol, bcol):
                # gst cols: s*2   = mean_s, s*2+1 = E2_s  (s = core)
                g3 = gst[:].rearrange("p (s j) -> p j s", s=8)   # [128,2,8]
                S2 = bnv[:, 0:2]
                nc.vector.reduce_sum(out=S2.rearrange("p (j a) -> p j a", j=2),
                                     in_=g3, axis=AX.X)
                nc.vector.tensor_scalar_mul(S2, S2, 1.0 / NCORES)
                gm = bnv[:, 0:1]
                ge2 = bnv[:, 1:2]
                # var = E[x^2] - mean^2
                nc.vector.tensor_mul(bnv[:, 2:3], gm, gm)
                nc.vector.tensor_sub(bnv[:, 2:3], ge2, bnv[:, 2:3])
                # inv_std = 1 / sqrt(var + eps)
                nc.scalar.activation(bnv[:, 3:4], bnv[:, 2:3], Act.Sqrt,
                                     bias=vecs_sb[:, 9:10])
                nc.vector.reciprocal(bnv[:, 4:5], bnv[:, 3:4])
                # scale = w * inv_std ; shift = b - mean * scale
                nc.vector.tensor_mul(bnv[:, 5:6], bnv[:, 4:5],
                                     vecs_sb[:, wcol:wcol + 1])
                nc.vector.tensor_mul(bnv[:, 6:7], gm, bnv[:, 5:6])
                nc.vector.tensor_sub(bnv[:, 7:8], vecs_sb[:, bcol:bcol + 1],
                                     bnv[:, 6:7])

            bnv1 = P([128, 8], f32, "bnv1")
            bnv2 = P([128, 8], f32, "bnv2")

            # ---- BN1 ----
            bn_sync(bns1, gst1, 8, 0, "cc1")
            # PE warmers: keep the p-state up through the collective. They
            # have no consumers; ~110 of them fit well inside the CC window.
            for w in range(97):
                pw = ps.tile([128, 512], f32, tag="mm", bufs=2, name=f"pw{w}")
                nc.tensor.matmul(pw[:], lhsT=fW1_sb[:, 0:128], rhs=fW2_sb[:],
                                 start=True, stop=True)
            bn_affine(gst1, bnv1, 1, 2)
            # FFN1 on *unnormalized* h1b: fold the BN1 scale into fW1's
            # partitions (fW1s = s*fW1) and the shift into the relu bias
            # (fbias = fW1^T t + ffb1).
            nc.vector.tensor_scalar(out=fW1s[:], in0=fW1_sb[:],
                                    scalar1=bnv1[:, 5:6], scalar2=None,
                                    op0=Alu.mult)
            tshift = sb.tile([128, 1], bf16, tag="tshift", bufs=1,
                             name="tshift")
            nc.vector.tensor_copy(tshift[:], bnv1[:, 7:8])
            pb = ps.tile([128, 4], f32, tag="av", bufs=2, name="pbias")
            for qf in range(4):
                nc.tensor.matmul(pb[:, qf:qf + 1],
                                 lhsT=fW1_sb[:, 128 * qf:128 * (qf + 1)],
                                 rhs=tshift[:], start=True, stop=True)
            nc.vector.tensor_add(fbias[:], pb[:], vecs_sb[:, 3:7])

            # ---- FFN (c-chunk pipelined; ffb2 cancels inside BN2) ----
            def emit_f1(c):
                for qp in range(2):
                    pf = ps.tile([128, 1024], f32, tag="sc", bufs=2,
                                 name=f"pf{c}{qp}")
                    for j in range(2):
                        qf = 2 * qp + j
                        nc.tensor.matmul(pf[:, 512 * j:512 * (j + 1)],
                                         lhsT=fW1s[:, 128 * qf:128 * (qf + 1)],
                                         rhs=h1b[:, 512 * c:512 * (c + 1)],
                                         start=True, stop=True)
                    for j in range(2):
                        qf = 2 * qp + j
                        if not (qp == 1 and j == 1):
                            nc.scalar.activation(
                                h2T[qf][:, 512 * c:512 * (c + 1)],
                                pf[:, 512 * j:512 * (j + 1)], Act.Relu,
                                bias=fbias[:, qf:qf + 1])
                        else:
                            nc.vector.tensor_scalar(
                                out=h2T[qf][:, 512 * c:512 * (c + 1)],
                                in0=pf[:, 512 * j:512 * (j + 1)],
                                scalar1=fbias[:, qf:qf + 1],
                                scalar2=0.0, op0=Alu.add, op1=Alu.max)

            def emit_h1n(c):
                nc.gpsimd.tensor_scalar(
                    out=h1nb[:, 512 * c:512 * (c + 1)],
                    in0=h1T[:, 512 * c:512 * (c + 1)],
                    scalar1=bnv1[:, 5:6], scalar2=bnv1[:, 7:8],
                    op0=Alu.mult, op1=Alu.add)

            def emit_f2(c):
                p2 = ps.tile([128, 512], f32, tag="av", bufs=2, name=f"p2{c}")
                for qf in range(4):
                    nc.tensor.matmul(p2[:],
                                     lhsT=fW2_sb[:, 128 * qf:128 * (qf + 1)],
                                     rhs=h2T[qf][:, 512 * c:512 * (c + 1)],
                                     start=(qf == 0), stop=(qf == 3))
                nc.vector.tensor_add(yT[:, 512 * c:512 * (c + 1)], p2[:],
                                     h1nb[:, 512 * c:512 * (c + 1)])
                nc.vector.bn_stats(bns2[:, 6 * c:6 * (c + 1)],
                                   yT[:, 512 * c:512 * (c + 1)])

            emit_f1(0)
            emit_h1n(0)
            emit_f1(1)
            emit_h1n(1)
            emit_f2(0)
            emit_f1(2)
            emit_h1n(2)
            emit_f2(1)
            emit_f1(3)
            emit_h1n(3)
            emit_f2(2)
            emit_f2(3)

            # ---- BN2 + output ----
            bn_sync(bns2, gst2, 10, 1, "cc2")
            bn_affine(gst2, bnv2, 7, 8)
            for c in range(4):
                nc.vector.tensor_scalar(
                    out=h1T[:, 512 * c:512 * (c + 1)],
                    in0=yT[:, 512 * c:512 * (c + 1)],
                    scalar1=bnv2[:, 5:6], scalar2=bnv2[:, 7:8],
                    op0=Alu.mult, op1=Alu.add)
                nc.sync.dma_start(d_yT[:, 512 * c:512 * (c + 1)],
                                  h1T[:, 512 * c:512 * (c + 1)])

    nc.compile()
    return nc


def _host_prep(inputs):
    f = np.float32
    Wq, Wk, Wv, Wo = (np.asarray(inputs[k], f) for k in ("Wq", "Wk", "Wv", "Wo"))
    WqQ = np.zeros((2, E, 128), f)
    WkQ = np.zeros((2, E, 128), f)
    WoQ = np.zeros((2, 128, E), f)
    for g in range(2):
        for hh in range(4):
            h = 4 * g + hh
            WqQ[g, :, 32 * hh:32 * hh + 16] = Wq[h]
            WkQ[g, :, 32 * hh:32 * hh + 16] = Wk[h]
            WoQ[g, 32 * hh + 1:32 * hh + 17, :] = Wo[h]
    WvI = np.ascontiguousarray(np.transpose(Wv, (1, 0, 2)).reshape(E, H * KD))
    fW2 = np.ascontiguousarray(
        np.asarray(inputs["ffW2"], f).reshape(4, 128, E).transpose(1, 0, 2))
    WeA = np.zeros((3, E), f)
    WeA[0:2] = np.asarray(inputs["We1"], f)
    WeA[2] = np.asarray(inputs["be1"], f)
    Eblk = np.zeros((128, 128), f)
    for h in range(4):
        Eblk[32 * h, 32 * h:32 * h + 32] = 1.0
    vecs = np.zeros((128, 12), f)
    vecs[:, 1] = inputs["bn1_w"]
    vecs[:, 2] = inputs["bn1_b"]
    vecs[:, 3:7] = np.asarray(inputs["ffb1"], f).reshape(4, 128).T
    vecs[:, 7] = inputs["bn2_w"]
    vecs[:, 8] = inputs["bn2_b"]
    vecs[:, 9] = EPS
    import ml_dtypes
    bf = ml_dtypes.bfloat16
    return {
        "WeA": WeA.astype(bf),
        "WqQ": np.ascontiguousarray(np.concatenate([WqQ[0], WqQ[1]], axis=1)).astype(bf),
        "WkQ": np.ascontiguousarray(np.concatenate([WkQ[0], WkQ[1]], axis=1)).astype(bf),
        "WvI": WvI.astype(bf),
        "WoQ": np.ascontiguousarray(np.concatenate([WoQ[0], WoQ[1]], axis=1)).astype(bf),
        "fW1": np.ascontiguousarray(np.asarray(inputs["ffW1"], f)).astype(bf),
        "fW2": np.ascontiguousarray(fW2.reshape(128, 512)).astype(bf),
        "Eblk": Eblk.astype(bf), "vecs": vecs,
    }


def _get_runner():
    """Build the sharded jitted executable once and cache it."""
    if "runner" in _CACHE:
        return _CACHE["runner"]
    import jax
    import concourse.mybir as mybir
    from jax.sharding import Mesh, PartitionSpec
    from jax.experimental.shard_map import shard_map
    from concourse.bass2jax import (_bass_exec_p, install_neuronx_cc_hook,
                                    partition_id_tensor)

    if "nc" not in _CACHE:
        _CACHE["nc"] = _build_nc()
    nc = _CACHE["nc"]
    install_neuronx_cc_hook()
    assert nc.dbg_addr is None

    partition_name = (nc.partition_id_tensor.name
                      if nc.partition_id_tensor else None)
    in_names, out_names, out_avals, zero_outs = [], [], [], []
    for alloc in nc.m.functions[0].allocations:
        if not isinstance(alloc, mybir.MemoryLocationSet):
            continue
        name = alloc.memorylocations[0].name
        if alloc.kind == "ExternalInput":
            if name != partition_name:
                in_names.append(name)
        elif alloc.kind == "ExternalOutput":
            shape = tuple(alloc.tensor_shape)
            dtype = mybir.dt.np(alloc.dtype)
            out_names.append(name)
            out_avals.append(jax.core.ShapedArray(shape, dtype))
            zero_outs.append(np.zeros(shape, dtype))
    n_params = len(in_names)
    n_outs = len(out_avals)
    all_in_names = list(in_names) + list(out_names)
    if partition_name is not None:
        all_in_names.append(partition_name)
    donate = tuple(range(n_params, n_params + n_outs))

    def _body(*args):
        operands = list(args)
        if partition_name is not None:
            operands.append(partition_id_tensor())
        outs = _bass_exec_p.bind(
            *operands,
            out_avals=tuple(out_avals),
            in_names=tuple(all_in_names),
            out_names=tuple(out_names),
            lowering_input_output_aliases=(),
            sim_require_finite=True,
            sim_require_nnan=True,
            nc=nc,
        )
        return tuple(outs)

    devices = jax.devices()[:NCORES]
    mesh = Mesh(np.asarray(devices), ("core",))
    in_specs = (PartitionSpec("core"),) * (n_params + n_outs)
    out_specs = (PartitionSpec("core"),) * len(out_names)
    sharded = jax.jit(
        shard_map(_body, mesh=mesh, in_specs=in_specs, out_specs=out_specs,
                  check_rep=False),
        donate_argnums=donate, keep_unused=True)

    def run(in_maps):
        per_core = [[np.asarray(m[name]) for name in in_names]
                    for m in in_maps]
        concat_in = [np.concatenate([per_core[c][i] for c in range(NCORES)],
                                    axis=0) for i in range(n_params)]
        concat_zeros = [np.zeros((NCORES * z.shape[0], *z.shape[1:]), z.dtype)
                        for z in zero_outs]
        out_arrs = sharded(*concat_in, *concat_zeros)
        out_arrs = [np.asarray(a) for a in out_arrs]
        return [{name: out_arrs[i].reshape(NCORES, *out_avals[i].shape)[c]
                 for i, name in enumerate(out_names)}
                for c in range(NCORES)]

    _CACHE["runner"] = run
    return run


def _make_in_maps(inputs):
    shared = _host_prep(inputs)
    x1 = np.asarray(inputs["x1"], np.float32)
    in_maps = []
    for cidx in range(NCORES):
        m = dict(shared)
        xl = x1[BPC * cidx:BPC * (cidx + 1)].reshape(T, D_IN)
        xa = np.ones((3, T), np.float32)
        xa[0:2] = xl.T
        import ml_dtypes
        m["xA"] = xa.astype(ml_dtypes.bfloat16)
        in_maps.append(m)
    return in_maps


def kernel(**inputs):
    run = _get_runner()
    results = run(_make_in_maps(inputs))
    outs = []
    for cidx in range(NCORES):
        yTo = results[cidx]["yT"]          # [E, T]
        outs.append(np.ascontiguousarray(yTo.T).reshape(BPC, N, E))
    return np.concatenate(outs, 0).astype(np.float32)


# revision 55
# speedup vs baseline: 1.0087x; 1.0087x over previous
"""Trainium2 Bass kernel for nn_MHABlock (dense transformer block).

Sharding: data-parallel over batch — 8 cores x 4 batches (2048 tokens/core).
BatchNorm stats are exact via two cross-core AllGathers of per-core
(mean, var) pairs ([128,2] each) + a local reduction.

On-device layout is E-major ("hT" = [E(128 partitions), tokens(free)]).
Attention uses the "scoresT" formulation (scores transposed: [k_tok, q_tok]):
softmax sums arrive free from an all-ones column prepended to V (row 32*hh of
the attnV PSUM = sum_k exp). Scores run in float32r (rounded into qT/kT by
the PSUM-evacuation copies); embedding/q/k/v/attnV/FFN in bf16. Softmax
normalization broadcasts 1/sum across each head's 32-partition group with a
single block-diagonal matmul (E_blk) + one tensor_tensor multiply; V_aug pad
columns are 1.0 so every reciprocal stays finite. Phase A is drizzled into
the attention stages (stage b only needs q/k chunk b + V tiles 4b..4b+3);
attnV lags scores by one chunk so the PE queue never head-of-line blocks on
exp. BN stats use bn_stats/bn_aggr per 512-token chunk overlapped with
attention/FFN; cross-core reduction is an AllGather of (mean, E[x^2]) pairs
+ local reduce (cheaper than AllReduce in both model and practice). Dummy
"warmer" matmuls keep the PE p-state up through the BN1 collective, and the
BN1 scale is folded into fW1 (fW1s) so FFN1 starts straight from raw h1.
"""

import numpy as np

B, N, D_IN, E, H, KD, FF = 32, 512, 2, 128, 8, 16, 512
NCORES = 8
BPC = B // NCORES          # batches per core
T = BPC * N                # 2048 local tokens
NTOK = B * N               # global token count for BN
NORM = 1.0 / np.sqrt(16.0)
EPS = 1e-5

_CACHE = {}
LAST_RESULT = None


def _build_nc():
    import concourse.bass as bass  # noqa: F401
    import concourse.mybir as mybir
    import concourse.tile as tile
    from concourse import bacc

    f32 = mybir.dt.float32
    f32r = mybir.dt.float32r
    bf16 = mybir.dt.bfloat16
    Act = mybir.ActivationFunctionType
    Alu = mybir.AluOpType
    AX = mybir.AxisListType

    nc = bacc.Bacc("TRN2", target_bir_lowering=False, debug=False,
                   enable_asserts=False, num_devices=NCORES)

    # ---- DRAM I/O ----
    d_xA = nc.dram_tensor("xA", [3, T], bf16, kind="ExternalInput").ap()
    d_WeA = nc.dram_tensor("WeA", [3, E], bf16, kind="ExternalInput").ap()
    d_Wb = nc.dram_tensor("Wb", [128, 2048], bf16, kind="ExternalInput").ap()
    d_vecs = nc.dram_tensor("vecs", [128, 12], f32, kind="ExternalInput").ap()
    d_yT = nc.dram_tensor("yT", [E, T], f32, kind="ExternalOutput").ap()

    RG = [list(range(NCORES))]

    with tile.TileContext(nc) as tc:
        with tc.sbuf_pool(name="sb", bufs=1) as sb, \
             tc.psum_pool(name="ps", bufs=1) as ps, \
             tc.tile_pool(name="dr", bufs=1, space="DRAM") as dr:

            def P(shape, dt, name):  # persistent tile
                return sb.tile(shape, dt, name=name, tag=name, bufs=1)

            xA = P([3, T], bf16, "xA_sb")
            WeA_sb = P([3, E], bf16, "WeA_sb")
            Wb_sb = P([128, 2048], bf16, "Wb_sb")
            WqQ_sb = Wb_sb[:, 0:256]
            WkQ_sb = Wb_sb[:, 256:512]
            WvI_sb = Wb_sb[:, 512:640]
            WoQ_sb = Wb_sb[:, 640:896]
            fW1_sb = Wb_sb[:, 896:1408]
            fW2_sb = Wb_sb[:, 1408:1920]
            Eblk_sb = Wb_sb[:, 1920:2048]
            Eblkf_sb = P([128, 128], f32, "Eblkf_sb")
            vecs_sb = P([128, 12], f32, "vecs_sb")

            H0b = P([128, T], bf16, "H0b")
            qT = [P([128, T], f32r, f"qT{g}") for g in range(2)]
            kT = [P([128, T], f32r, f"kT{g}") for g in range(2)]
            V_aug = P([128, 16 * 256], bf16, "V_aug")
            h1T = P([128, T], f32, "h1T")
            h1b = P([128, T], bf16, "h1b")
            h1nb = P([128, T], bf16, "h1nb")
            fW1s = P([128, FF], bf16, "fW1s")
            fbias = P([128, 4], f32, "fbias")
            h2T = [P([128, T], bf16, f"h2T{qf}") for qf in range(4)]
            yT = P([128, T], f32, "yT_sb")
            recs = P([128, 512], f32, "recs")
            recsb = P([128, 512], bf16, "recsb")
            bns1 = P([128, 30], f32, "bns1")
            bns2 = P([128, 30], f32, "bns2")
            gst1 = P([128, 16], f32, "gst1")
            gst2 = P([128, 16], f32, "gst2")
            st = P([128, 12], f32, "st")    # scratch for BN math

            # ---- load inputs ----
            nc.sync.dma_start(xA[:], d_xA)
            nc.sync.dma_start(WeA_sb[:], d_WeA)
            nc.sync.dma_start(Wb_sb[:, 0:640], d_Wb[:, 0:640])
            nc.sync.dma_start(Wb_sb[:, 640:2048], d_Wb[:, 640:2048])
            nc.gpsimd.tensor_copy(Eblkf_sb[:], Eblk_sb)
            nc.sync.dma_start(vecs_sb[:], d_vecs)
            nc.gpsimd.memset(V_aug[:], 0.0)
            # ones at w=0 (softmax sums) and w=17..31 (keeps reciprocals finite)
            va = V_aug.rearrange("p (t h w) -> p (t h) w", t=16, h=8)
            nc.gpsimd.memset(va[:, :, 0:1], 1.0)
            nc.gpsimd.memset(va[:, :, 17:32], 1.0)

            # ---- Phase A helpers (emitted piecewise, interleaved with B) ----
            def emit_embed(c):
                pm = ps.tile([128, 512], f32, tag="mm", bufs=2, name=f"pe{c}")
                nc.tensor.matmul(pm[:], lhsT=WeA_sb[:],
                                 rhs=xA[:, 512 * c:512 * (c + 1)],
                                 start=True, stop=True)
                nc.vector.tensor_copy(H0b[:, 512 * c:512 * (c + 1)], pm[:])

            def emit_v(t):
                pv = ps.tile([128, 128], f32, tag="mm", bufs=2, name=f"pv{t}")
                nc.tensor.matmul(pv[:], lhsT=H0b[:, 128 * t:128 * (t + 1)],
                                 rhs=WvI_sb, start=True, stop=True)
                dst = V_aug[:, 256 * t:256 * (t + 1)]
                dst = dst.rearrange("p (h w) -> p h w", h=8)[:, :, 1:17]
                src = pv.rearrange("p (h w) -> p h w", h=8)
                nc.vector.tensor_copy(dst, src)

            def emit_qk_chunk(g, c):
                # q/k chunks go through the "mm" ring to stay out of the
                # attention pipeline's "sc" ring
                for W, dstT in ((WqQ_sb, qT), (WkQ_sb, kT)):
                    pq = ps.tile([128, 512], f32, tag="mm", bufs=2,
                                 name=f"pqk{g}{c}_{0 if W is WqQ_sb else 1}")
                    nc.tensor.matmul(pq[:],
                                     lhsT=W[:, 128 * g:128 * (g + 1)],
                                     rhs=H0b[:, 512 * c:512 * (c + 1)],
                                     start=True, stop=True)
                    nc.vector.tensor_copy(
                        dstT[g][:, 512 * c:512 * (c + 1)], pq[:])

            # minimal prefix needed to start attention on (b0, g0/g1):
            # stage b only touches q/k chunk b and V tiles 4b..4b+3
            emit_embed(0)
            emit_qk_chunk(0, 0)
            emit_qk_chunk(1, 0)
            for t in range(4):
                emit_v(t)
            # deferred phase-A work, drizzled into the attention stages
            deferred_a = []
            for c in range(1, 4):
                deferred_a.append(lambda c=c: emit_embed(c))
                deferred_a.append(lambda c=c: emit_qk_chunk(0, c))
                deferred_a.append(lambda c=c: emit_qk_chunk(1, c))
                deferred_a.extend(lambda t=t: emit_v(t)
                                  for t in range(4 * c, 4 * c + 4))

            # ---- Phase B: attention (software-pipelined stages) ----
            HT = {}

            def emit_norm(b, g, av, last=False):
                nc.vector.reciprocal_approx_fast(recs[:], av[:])
                if not last:
                    nc.gpsimd.tensor_copy(recsb[:], recs[:])
                raw = sb.tile([128, 512], f32, tag="raw", bufs=2,
                              name=f"raw{b}{g}")
                nc.vector.tensor_copy(raw[:], av[:])
                Rp = ps.tile([128, 512], f32, tag="mm", bufs=2,
                             name=f"Rp{b}{g}")
                if last:
                    # fp32 matmul (4 cyc/row) but skips the Pool-hop latency —
                    # this one sits on the BN1 critical tail where PE is idle
                    nc.tensor.matmul(Rp[:], lhsT=Eblkf_sb[:], rhs=recs[:],
                                     start=True, stop=True)
                else:
                    nc.tensor.matmul(Rp[:], lhsT=Eblk_sb, rhs=recsb[:],
                                     start=True, stop=True)
                HT[(b, g)] = sb.tile([128, 512], bf16, tag=f"HT{g}", bufs=2,
                                     name=f"HT{g}_{b}")
                nc.vector.tensor_mul(HT[(b, g)][:], raw[:], Rp[:])

            def emit_po(b):
                po = ps.tile([128, 512], f32, tag="mm", bufs=2, name=f"po{b}")
                for g in range(2):
                    nc.tensor.matmul(po[:],
                                     lhsT=WoQ_sb[:, 128 * g:128 * (g + 1)],
                                     rhs=HT[(b, g)][:],
                                     start=(g == 0), stop=(g == 1))
                nc.vector.tensor_add(h1T[:, 512 * b:512 * (b + 1)], po[:],
                                     H0b[:, 512 * b:512 * (b + 1)])
                nc.vector.bn_stats(bns1[:, 6 * b:6 * (b + 1)],
                                   h1T[:, 512 * b:512 * (b + 1)])
                nc.vector.tensor_copy(h1b[:, 512 * b:512 * (b + 1)],
                                      h1T[:, 512 * b:512 * (b + 1)])

            stages = [(b, g) for b in range(4) for g in range(2)]
            pending = []          # [(kind, args), ...] from previous stage
            for k, (b, g) in enumerate(stages):
                av = ps.tile([128, 512], f32, tag="av", bufs=2,
                             name=f"av{b}{g}")
                def emit_av(c, exs):
                    for hp in range(2):
                        for j in range(2):
                            hh = 2 * hp + j
                            h = 4 * g + hh
                            tci = 4 * b + c
                            nc.tensor.matmul(
                                av[32 * hh:32 * (hh + 1), :],
                                lhsT=V_aug[:, 256 * tci + 32 * h:
                                           256 * tci + 32 * h + 32],
                                rhs=exs[hp][:, 512 * j:512 * (j + 1)],
                                start=(c == 0), stop=(c == 3),
                                tile_position=(0, 32 * hh))

                exq = {}
                for c in range(4):
                    for hp in range(2):
                        scp = ps.tile([128, 1024], f32, tag="sc", bufs=2,
                                      name=f"scp{b}{g}{c}{hp}")
                        for j in range(2):
                            hh = 2 * hp + j
                            nc.tensor.matmul(
                                scp[:, 512 * j:512 * (j + 1)],
                                lhsT=kT[g][32 * hh:32 * (hh + 1),
                                           512 * b + 128 * c:
                                           512 * b + 128 * (c + 1)],
                                rhs=qT[g][32 * hh:32 * (hh + 1),
                                          512 * b:512 * (b + 1)],
                                start=True, stop=True,
                                tile_position=(32 * hh, 0))
                        ex = sb.tile([128, 1024], bf16, tag="ex", bufs=6,
                                     name=f"ex{b}{g}{c}{hp}")
                        nc.scalar.activation(ex[:], scp[:], Act.Exp,
                                             scale=float(NORM))
                        exq[(c, hp)] = ex
                    # attnV for the PREVIOUS chunk: keeps the next chunk's
                    # score MMs ahead of the exp dependency in the PE queue
                    if c >= 1:
                        emit_av(c - 1, (exq[(c - 1, 0)], exq[(c - 1, 1)]))
                    if c == 1:
                        for fn in pending:
                            fn()
                        pending = []
                    if c in (1, 2):
                        for _ in range(2):
                            if deferred_a:
                                deferred_a.pop(0)()
                emit_av(3, (exq[(3, 0)], exq[(3, 1)]))
                # queue this stage's normalization (and po once g==1 done)
                pending = [lambda b=b, g=g, av=av, lt=(k == len(stages) - 1):
                           emit_norm(b, g, av, last=lt)]
                if g == 1:
                    pending.append(lambda b=b: emit_po(b))
            for fn in pending:
                fn()
            for fn in deferred_a:
                fn()

            # ---- BN sync helper: AllGather of per-core (mean, E[x^2]) ----
            def bn_sync(bns, gst, agcol, scol, ccname):
                agg = st[:, agcol:agcol + 2]
                nc.vector.bn_aggr(agg, bns[:])
                scr = st[:, scol:scol + 1]
                nc.vector.tensor_mul(scr, agg[:, 0:1], agg[:, 0:1])
                nc.vector.tensor_add(agg[:, 1:2], agg[:, 1:2], scr)
                cc_in = dr.tile([128, 2], f32, name=f"{ccname}_in",
                                tag=f"{ccname}_in")
                cc_out = dr.tile([1024, 2], f32, addr_space="Shared",
                                 name=f"{ccname}_out", tag=f"{ccname}_out")
                nc.sync.dma_start(cc_in[:], agg)
                nc.gpsimd.collective_compute(
                    "AllGather", Alu.bypass, replica_groups=RG,
                    ins=[cc_in[:]], outs=[cc_out[:]])
                src = cc_out[:].rearrange("(s p) j -> p s j", s=8)
                dst = gst[:].rearrange("p (s j) -> p s j", s=8)
                nc.sync.dma_start(dst, src)

            def bn_affine(gst, bnv, wcol, bcol):
                # gst cols: s*2   = mean_s, s*2+1 = E2_s  (s = core)
                g3 = gst[:].rearrange("p (s j) -> p j s", s=8)   # [128,2,8]
                S2 = bnv[:, 0:2]
                nc.vector.reduce_sum(out=S2.rearrange("p (j a) -> p j a", j=2),
                                     in_=g3, axis=AX.X)
                nc.vector.tensor_scalar_mul(S2, S2, 1.0 / NCORES)
                gm = bnv[:, 0:1]
                ge2 = bnv[:, 1:2]
                # var = E[x^2] - mean^2
                nc.vector.tensor_mul(bnv[:, 2:3], gm, gm)
                nc.vector.tensor_sub(bnv[:, 2:3], ge2, bnv[:, 2:3])
                # inv_std = 1 / sqrt(var + eps)
                nc.scalar.activation(bnv[:, 3:4], bnv[:, 2:3], Act.Sqrt,
                                     bias=vecs_sb[:, 9:10])
                nc.vector.reciprocal(bnv[:, 4:5], bnv[:, 3:4])
                # scale = w * inv_std ; shift = b - mean * scale
                nc.vector.tensor_mul(bnv[:, 5:6], bnv[:, 4:5],
                                     vecs_sb[:, wcol:wcol + 1])
                nc.vector.tensor_mul(bnv[:, 6:7], gm, bnv[:, 5:6])
                nc.vector.tensor_sub(bnv[:, 7:8], vecs_sb[:, bcol:bcol + 1],
                                     bnv[:, 6:7])

            bnv1 = P([128, 8], f32, "bnv1")
            bnv2 = P([128, 8], f32, "bnv2")

            # ---- BN1 ----
            bn_sync(bns1, gst1, 8, 0, "cc1")
            # PE warmers: keep the p-state up through the collective. They
            # have no consumers; ~110 of them fit well inside the CC window.
            for w in range(97):
                pw = ps.tile([128, 512], f32, tag="mm", bufs=2, name=f"pw{w}")
                nc.tensor.matmul(pw[:], lhsT=fW1_sb[:, 0:128], rhs=fW2_sb,
                                 start=True, stop=True)
            bn_affine(gst1, bnv1, 1, 2)
            # FFN1 on *unnormalized* h1b: fold the BN1 scale into fW1's
            # partitions (fW1s = s*fW1) and the shift into the relu bias
            # (fbias = fW1^T t + ffb1).
            nc.vector.tensor_scalar(out=fW1s[:], in0=fW1_sb,
                                    scalar1=bnv1[:, 5:6], scalar2=None,
                                    op0=Alu.mult)
            tshift = sb.tile([128, 1], bf16, tag="tshift", bufs=1,
                             name="tshift")
            nc.vector.tensor_copy(tshift[:], bnv1[:, 7:8])
            pb = ps.tile([128, 4], f32, tag="av", bufs=2, name="pbias")
            for qf in range(4):
                nc.tensor.matmul(pb[:, qf:qf + 1],
                                 lhsT=fW1_sb[:, 128 * qf:128 * (qf + 1)],
                                 rhs=tshift[:], start=True, stop=True)
            nc.vector.tensor_add(fbias[:], pb[:], vecs_sb[:, 3:7])

            # ---- FFN (c-chunk pipelined; ffb2 cancels inside BN2) ----
            def emit_f1(c):
                for qp in range(2):
                    pf = ps.tile([128, 1024], f32, tag="sc", bufs=2,
                                 name=f"pf{c}{qp}")
                    for j in range(2):
                        qf = 2 * qp + j
                        nc.tensor.matmul(pf[:, 512 * j:512 * (j + 1)],
                                         lhsT=fW1s[:, 128 * qf:128 * (qf + 1)],
                                         rhs=h1b[:, 512 * c:512 * (c + 1)],
                                         start=True, stop=True)
                    for j in range(2):
                        qf = 2 * qp + j
                        if not (qp == 1 and j == 1):
                            nc.scalar.activation(
                                h2T[qf][:, 512 * c:512 * (c + 1)],
                                pf[:, 512 * j:512 * (j + 1)], Act.Relu,
                                bias=fbias[:, qf:qf + 1])
                        else:
                            nc.vector.tensor_scalar(
                                out=h2T[qf][:, 512 * c:512 * (c + 1)],
                                in0=pf[:, 512 * j:512 * (j + 1)],
                                scalar1=fbias[:, qf:qf + 1],
                                scalar2=0.0, op0=Alu.add, op1=Alu.max)

            def emit_h1n(c):
                nc.gpsimd.tensor_scalar(
                    out=h1nb[:, 512 * c:512 * (c + 1)],
                    in0=h1T[:, 512 * c:512 * (c + 1)],
                    scalar1=bnv1[:, 5:6], scalar2=bnv1[:, 7:8],
                    op0=Alu.mult, op1=Alu.add)

            def emit_f2(c):
                p2 = ps.tile([128, 512], f32, tag="av", bufs=2, name=f"p2{c}")
                for qf in range(4):
                    nc.tensor.matmul(p2[:],
                                     lhsT=fW2_sb[:, 128 * qf:128 * (qf + 1)],
                                     rhs=h2T[qf][:, 512 * c:512 * (c + 1)],
                                     start=(qf == 0), stop=(qf == 3))
                nc.vector.tensor_add(yT[:, 512 * c:512 * (c + 1)], p2[:],
                                     h1nb[:, 512 * c:512 * (c + 1)])
                nc.vector.bn_stats(bns2[:, 6 * c:6 * (c + 1)],
                                   yT[:, 512 * c:512 * (c + 1)])

            emit_f1(0)
            emit_h1n(0)
            emit_f1(1)
            emit_h1n(1)
            emit_f2(0)
            emit_f1(2)
            emit_h1n(2)
            emit_f2(1)
            emit_f1(3)
            emit_h1n(3)
            emit_f2(2)
            emit_f2(3)

            # ---- BN2 + output ----
            bn_sync(bns2, gst2, 10, 1, "cc2")
            bn_affine(gst2, bnv2, 7, 8)
            for c in range(4):
                nc.vector.tensor_scalar(
                    out=h1T[:, 512 * c:512 * (c + 1)],
                    in0=yT[:, 512 * c:512 * (c + 1)],
                    scalar1=bnv2[:, 5:6], scalar2=bnv2[:, 7:8],
                    op0=Alu.mult, op1=Alu.add)
                nc.sync.dma_start(d_yT[:, 512 * c:512 * (c + 1)],
                                  h1T[:, 512 * c:512 * (c + 1)])

    nc.compile()
    return nc


def _host_prep(inputs):
    f = np.float32
    Wq, Wk, Wv, Wo = (np.asarray(inputs[k], f) for k in ("Wq", "Wk", "Wv", "Wo"))
    WqQ = np.zeros((2, E, 128), f)
    WkQ = np.zeros((2, E, 128), f)
    WoQ = np.zeros((2, 128, E), f)
    for g in range(2):
        for hh in range(4):
            h = 4 * g + hh
            WqQ[g, :, 32 * hh:32 * hh + 16] = Wq[h]
            WkQ[g, :, 32 * hh:32 * hh + 16] = Wk[h]
            WoQ[g, 32 * hh + 1:32 * hh + 17, :] = Wo[h]
    WvI = np.ascontiguousarray(np.transpose(Wv, (1, 0, 2)).reshape(E, H * KD))
    fW2 = np.ascontiguousarray(
        np.asarray(inputs["ffW2"], f).reshape(4, 128, E).transpose(1, 0, 2))
    WeA = np.zeros((3, E), f)
    WeA[0:2] = np.asarray(inputs["We1"], f)
    WeA[2] = np.asarray(inputs["be1"], f)
    Eblk = np.zeros((128, 128), f)
    for h in range(4):
        Eblk[32 * h, 32 * h:32 * h + 32] = 1.0
    vecs = np.zeros((128, 12), f)
    vecs[:, 1] = inputs["bn1_w"]
    vecs[:, 2] = inputs["bn1_b"]
    vecs[:, 3:7] = np.asarray(inputs["ffb1"], f).reshape(4, 128).T
    vecs[:, 7] = inputs["bn2_w"]
    vecs[:, 8] = inputs["bn2_b"]
    vecs[:, 9] = EPS
    import ml_dtypes
    bf = ml_dtypes.bfloat16
    Wb = np.concatenate([
        np.concatenate([WqQ[0], WqQ[1]], axis=1),
        np.concatenate([WkQ[0], WkQ[1]], axis=1),
        WvI,
        np.concatenate([WoQ[0], WoQ[1]], axis=1),
        np.asarray(inputs["ffW1"], f),
        fW2.reshape(128, 512),
        Eblk,
    ], axis=1)
    return {
        "WeA": WeA.astype(bf),
        "Wb": np.ascontiguousarray(Wb).astype(bf),
        "vecs": vecs,
    }


def _get_runner():
    """Build the sharded jitted executable once and cache it."""
    if "runner" in _CACHE:
        return _CACHE["runner"]
    import jax
    import concourse.mybir as mybir
    from jax.sharding import Mesh, PartitionSpec
    from jax.experimental.shard_map import shard_map
    from concourse.bass2jax import (_bass_exec_p, install_neuronx_cc_hook,
                                    partition_id_tensor)

    if "nc" not in _CACHE:
        _CACHE["nc"] = _build_nc()
    nc = _CACHE["nc"]
    install_neuronx_cc_hook()
    assert nc.dbg_addr is None

    partition_name = (nc.partition_id_tensor.name
                      if nc.partition_id_tensor else None)
    in_names, out_names, out_avals, zero_outs = [], [], [], []
    for alloc in nc.m.functions[0].allocations:
        if not isinstance(alloc, mybir.MemoryLocationSet):
            continue
        name = alloc.memorylocations[0].name
        if alloc.kind == "ExternalInput":
            if name != partition_name:
                in_names.append(name)
        elif alloc.kind == "ExternalOutput":
            shape = tuple(alloc.tensor_shape)
            dtype = mybir.dt.np(alloc.dtype)
            out_names.append(name)
            out_avals.append(jax.core.ShapedArray(shape, dtype))
            zero_outs.append(np.zeros(shape, dtype))
    n_params = len(in_names)
    n_outs = len(out_avals)
    all_in_names = list(in_names) + list(out_names)
    if partition_name is not None:
        all_in_names.append(partition_name)
    donate = tuple(range(n_params, n_params + n_outs))

    def _body(*args):
        operands = list(args)
        if partition_name is not None:
            operands.append(partition_id_tensor())
        outs = _bass_exec_p.bind(
            *operands,
            out_avals=tuple(out_avals),
            in_names=tuple(all_in_names),
            out_names=tuple(out_names),
            lowering_input_output_aliases=(),
            sim_require_finite=True,
            sim_require_nnan=True,
            nc=nc,
        )
        return tuple(outs)

    devices = jax.devices()[:NCORES]
    mesh = Mesh(np.asarray(devices), ("core",))
    in_specs = (PartitionSpec("core"),) * (n_params + n_outs)
    out_specs = (PartitionSpec("core"),) * len(out_names)
    sharded = jax.jit(
        shard_map(_body, mesh=mesh, in_specs=in_specs, out_specs=out_specs,
                  check_rep=False),
        donate_argnums=donate, keep_unused=True)

    def run(in_maps):
        per_core = [[np.asarray(m[name]) for name in in_names]
                    for m in in_maps]
        concat_in = [np.concatenate([per_core[c][i] for c in range(NCORES)],
                                    axis=0) for i in range(n_params)]
        concat_zeros = [np.zeros((NCORES * z.shape[0], *z.shape[1:]), z.dtype)
                        for z in zero_outs]
        out_arrs = sharded(*concat_in, *concat_zeros)
        out_arrs = [np.asarray(a) for a in out_arrs]
        return [{name: out_arrs[i].reshape(NCORES, *out_avals[i].shape)[c]
                 for i, name in enumerate(out_names)}
                for c in range(NCORES)]

    _CACHE["runner"] = run
    return run


def _make_in_maps(inputs):
    shared = _host_prep(inputs)
    x1 = np.asarray(inputs["x1"], np.float32)
    in_maps = []
    for cidx in range(NCORES):
        m = dict(shared)
        xl = x1[BPC * cidx:BPC * (cidx + 1)].reshape(T, D_IN)
        xa = np.ones((3, T), np.float32)
        xa[0:2] = xl.T
        import ml_dtypes
        m["xA"] = xa.astype(ml_dtypes.bfloat16)
        in_maps.append(m)
    return in_maps


def kernel(**inputs):
    run = _get_runner()
    results = run(_make_in_maps(inputs))
    outs = []
    for cidx in range(NCORES):
        yTo = results[cidx]["yT"]          # [E, T]
        outs.append(np.ascontiguousarray(yTo.T).reshape(BPC, N, E))
    return np.concatenate(outs, 0).astype(np.float32)
